# revision 1
# baseline (speedup 1.0000x reference)
"""Trainium2 Bass kernel for nn_Net_66975720014255 (gnn_message_passing).

Sharding: data-parallel over batch B=32 across 8 NeuronCores (4 batches per
core); adjacency and all weights replicated. No collectives.

Per-core device program (C=40, T=12, N=800, R=11):
  layouts per local batch b:
    x'  (non-T): rows q=(t,c) on partitions, node n on free   [480, 800]
    x'T (T):     node n on partitions, q=(t,c) on free        [800, 480]
  - tconv gates:  banded block matrix Wbig [480,440] (host-built) as lhsT,
                  rhs = x' tiles; tanh/sigmoid on ACT; product on DVE
  - hop0 h@adj:   lhsT = x'T column windows (2-tap window trick over the
                  2N-wide sliding window), rhs = adj rows; PSUM accumulate
  - mix1:         per <=128-node chunk: lhsT = hop0 rows, rhs = blockdiag(W1^T)
                  -> output lands transposed = h1^T, ready for hop1
  - hop1:         lhsT = h1^T, rhs = adj[:, 800:]
  - mix2:         lhsT = blockdiag(W2^T), rhs = h2 rows (f32r)
  - skip/resid:   banded block matrices over rows, BN_SCALE folded on host
Embedding adds, adj=relu(nv1@nv2), weight reshaping, BN folding: host numpy.
"""

import sys

if '/opt/trn_rl_repo' not in sys.path:
    sys.path.insert(0, '/opt/trn_rl_repo')

import numpy as np
import ml_dtypes

import concourse.bass as bass  # noqa: F401
import concourse.tile as tile
from concourse import bacc, mybir
from concourse.bass_utils import run_bass_kernel_spmd

# ----- problem constants (hardcoded per contract) -----
B, C, T, N = 32, 40, 12, 800
R = T - 1                    # 11
N2 = 2 * N                   # 1600
NCORES = 8
BL = B // NCORES             # 4 local batches per core
BN_SCALE = float(1.0 / np.sqrt(1.0 + 1e-5))

Q = T * C                    # 480 rows (t,c) per batch (non-T layout)
RQ = R * C                   # 440 rows (r,c) per batch
SQ = 12 * C                  # 480 skip rows (s,c) per batch

M_BLOCKS = [(0, 120), (120, 120), (240, 120), (360, 80)]          # (r,c) row blocks
K_BLOCKS_Q = [(0, 120), (120, 120), (240, 120), (360, 120)]       # (t,c) row blocks
# 1600-node split: 12x128 + 64 (13 dense K-tiles; windowing done on host)
N2_SPLIT = [(k * 128, 128) for k in range(12)] + [(1536, 64)]
CH800 = [(0, 400), (400, 400)]
CH1600 = [(0, 400), (400, 400), (800, 400), (1200, 400)]

F32 = mybir.dt.float32
F32R = mybir.dt.float32r
BF16 = mybir.dt.bfloat16

ADJ_BF16 = True              # bf16 for the big adjacency chain; else f32r
_np_bf16 = ml_dtypes.bfloat16


def _adj_np_dt():
    return _np_bf16 if ADJ_BF16 else np.float32


def _adj_dt():
    return BF16 if ADJ_BF16 else F32R


def _mm(x):
    """matmul operands are declared float32r end-to-end; no-op passthrough."""
    return x


# ---------------------------------------------------------------------------
# host-side preparation (pure numpy)
# ---------------------------------------------------------------------------

def _prep_weights(inp):
    f32 = np.float32
    nv1, nv2 = np.asarray(inp['nv1'], f32), np.asarray(inp['nv2'], f32)
    adj = np.maximum(f32(0), nv1 @ nv2)                       # (1600,1600)

    def wbig(W):
        Wb = np.zeros((Q, RQ), f32)
        W0, W1 = np.asarray(W[:, :, 0], f32), np.asarray(W[:, :, 1], f32)
        for r in range(R):
            Wb[r * C:(r + 1) * C, r * C:(r + 1) * C] = W0.T          # t == r
            Wb[(r + 1) * C:(r + 2) * C, r * C:(r + 1) * C] = W1.T    # t == r+1
        return Wb

    def blkdiag3(A):                                          # A is (c, d)
        M = np.zeros((120, 120), f32)
        for j in range(3):
            M[j * C:(j + 1) * C, j * C:(j + 1) * C] = A
        return M

    wmix1 = blkdiag3(np.asarray(inp['W_gcn'][0], f32).T).astype(_adj_np_dt())
    wmix2 = blkdiag3(np.asarray(inp['W_gcn'][1], f32).T)      # f32 (f32r matmul)

    eye = np.eye(C, dtype=f32)
    wskip = np.zeros((RQ, SQ), f32)
    Ws = np.asarray(inp['W_skip'], f32) * BN_SCALE            # (12, 11)
    bs = np.asarray(inp['b_skip'], f32) * BN_SCALE
    wskip_bias = np.zeros((1, SQ), f32)
    for s in range(12):
        for r in range(R):
            wskip[r * C:(r + 1) * C, s * C:(s + 1) * C] = Ws[s, r] * eye
        wskip_bias[0, s * C:(s + 1) * C] = bs[s]

    wres = np.zeros((Q, RQ), f32)
    Wr = np.asarray(inp['W_res'], f32) * BN_SCALE             # (11, 12)
    for t in range(T):
        for r in range(R):
            wres[t * C:(t + 1) * C, r * C:(r + 1) * C] = Wr[r, t] * eye

    bias_f = np.ascontiguousarray(np.tile(np.asarray(inp['b_f'], f32), 3)[:, None])
    bias_g = np.ascontiguousarray(np.tile(np.asarray(inp['b_g'], f32), 3)[:, None])

    bres = np.asarray(inp['b_res'], f32) * BN_SCALE           # (11,)
    bres_tile = np.zeros((120, 1), f32)
    for p in range(120):
        r = p // C
        bres_tile[p, 0] = bres[r] if r < R else 0.0

    return dict(adj=np.ascontiguousarray(adj.astype(_adj_np_dt())),
                wbig_f=wbig(np.asarray(inp['W_f'])),
                wbig_g=wbig(np.asarray(inp['W_g'])),
                wmix1=wmix1, wmix2=wmix2, wskip=wskip, wres=wres,
                wskip_bias=wskip_bias, has_bskip=bool(np.any(bs)),
                bias_f=bias_f, bias_g=bias_g,
                bres_tile=bres_tile, has_bres=bool(np.any(bres)))


def _prep_data(inp):
    f32 = np.float32
    x = np.asarray(inp['x'], f32) + np.asarray(inp['t_emb'], f32) \
        + np.asarray(inp['s_emb'], f32)                        # (B,C,T,N)
    xp = np.ascontiguousarray(x.transpose(0, 2, 1, 3)).reshape(B, Q, N)
    xpt = np.ascontiguousarray(x.transpose(0, 3, 2, 1)).reshape(B, N, Q)
    # windowed transpose: rows k in [0,800) -> x'[c, r, k]; k in [800,1600) ->
    # x'[c, r+1, k-800]; cols (r, c) = first 440 resp. last 440 of (t, c)
    wxt = np.concatenate([xpt[:, :, :RQ], xpt[:, :, C:]], axis=1)  # (B, 1600, 440)
    wxt = np.ascontiguousarray(wxt.astype(_adj_np_dt()))
    xp_cores = [np.ascontiguousarray(xp[i * BL:(i + 1) * BL]) for i in range(NCORES)]
    wxt_cores = [np.ascontiguousarray(wxt[i * BL:(i + 1) * BL]) for i in range(NCORES)]
    return xp_cores, wxt_cores


# ---------------------------------------------------------------------------
# device program
# ---------------------------------------------------------------------------

def _build_program(has_bres, has_bskip):
    nc = bacc.Bacc("TRN2", target_bir_lowering=False, debug=False,
                   enable_asserts=False, num_devices=NCORES)
    adt = _adj_dt()

    xp_d = nc.dram_tensor("xp", [BL, Q, N], F32R, kind="ExternalInput").ap()
    wxt_d = nc.dram_tensor("wxt", [BL, N2, RQ], adt, kind="ExternalInput").ap()
    adj_d = nc.dram_tensor("adj", [N2, N2], adt, kind="ExternalInput").ap()
    wbigf_d = nc.dram_tensor("wbig_f", [Q, RQ], F32R, kind="ExternalInput").ap()
    wbigg_d = nc.dram_tensor("wbig_g", [Q, RQ], F32R, kind="ExternalInput").ap()
    wmix1_d = nc.dram_tensor("wmix1", [120, 120], adt, kind="ExternalInput").ap()
    wmix2_d = nc.dram_tensor("wmix2", [120, 120], F32R, kind="ExternalInput").ap()
    wskip_d = nc.dram_tensor("wskip", [RQ, SQ], F32R, kind="ExternalInput").ap()
    wskipb_d = nc.dram_tensor("wskip_bias", [1, SQ], F32R, kind="ExternalInput").ap()
    wres_d = nc.dram_tensor("wres", [Q, RQ], F32R, kind="ExternalInput").ap()
    biasf_d = nc.dram_tensor("bias_f", [120, 1], F32, kind="ExternalInput").ap()
    biasg_d = nc.dram_tensor("bias_g", [120, 1], F32, kind="ExternalInput").ap()
    bres_d = nc.dram_tensor("bres", [120, 1], F32, kind="ExternalInput").ap()
    # output rows per batch: 0:440 final (r,c), 440:920 skip (s,c)
    out_d = nc.dram_tensor("out", [BL, 920, N], F32, kind="ExternalOutput").ap()

    with tile.TileContext(nc) as tc:
        _emit(nc, tc, xp_d, wxt_d, adj_d, wbigf_d, wbigg_d, wmix1_d, wmix2_d,
              wskip_d, wskipb_d, wres_d, biasf_d, biasg_d, bres_d, out_d,
              has_bres, has_bskip)
    nc.compile()
    return nc


def _emit(nc, tc, xp_d, wxt_d, adj_d, wbigf_d, wbigg_d, wmix1_d, wmix2_d,
          wskip_d, wskipb_d, wres_d, biasf_d, biasg_d, bres_d, out_d,
          has_bres, has_bskip):
    from contextlib import ExitStack
    adt = _adj_dt()
    AF = mybir.ActivationFunctionType
    ALU = mybir.AluOpType
    ctx = ExitStack()
    with ctx:
        const = ctx.enter_context(tc.tile_pool(name="const", bufs=1))
        # ---- pools ----
        xp_p = ctx.enter_context(tc.tile_pool(name="xp", bufs=2))
        xpt_p = ctx.enter_context(tc.tile_pool(name="xpt", bufs=2))
        dres_p = ctx.enter_context(tc.tile_pool(name="dres", bufs=1))
        hop0_p = ctx.enter_context(tc.tile_pool(name="hop0sb", bufs=3))
        h1t_p = ctx.enter_context(tc.tile_pool(name="h1t", bufs=1))
        h2_p = ctx.enter_context(tc.tile_pool(name="h2sb", bufs=2))
        oraw_p = ctx.enter_context(tc.tile_pool(name="oraw", bufs=1))
        tmp_p = ctx.enter_context(tc.tile_pool(name="tmp", bufs=2))
        fin_p = ctx.enter_context(tc.tile_pool(name="fin", bufs=4))
        psA = ctx.enter_context(tc.tile_pool(name="psA", bufs=6, space="PSUM"))
        psB = ctx.enter_context(tc.tile_pool(name="psB", bufs=2, space="PSUM"))

        # ---- DMA order: tconv-critical inputs first, bulk weights behind ----
        biasf_sb = const.tile([120, 1], F32, name="biasf")
        nc.sync.dma_start(biasf_sb[:], biasf_d[:])
        biasg_sb = const.tile([120, 1], F32, name="biasg")
        nc.scalar.dma_start(biasg_sb[:], biasg_d[:])
        wbig_sb = {}
        for gname, wd in (("f", wbigf_d), ("g", wbigg_d)):
            tiles = []
            for k, (o, s) in enumerate(K_BLOCKS_Q):
                t = const.tile([s, RQ], F32R, name=f"wbig{gname}{k}")
                eng = nc.sync if gname == "f" else nc.scalar
                eng.dma_start(t[:], wd[o:o + s, :])
                tiles.append(t)
            wbig_sb[gname] = tiles
        # adjacency on the gpsimd queue, parallel with everything above;
        # column-chunk-major so hop0 chain ch=0 starts after ~1/4 of the bytes
        adj_sb = []
        for i, (o, s) in enumerate(N2_SPLIT):
            adj_sb.append(const.tile([s, N2], adt, name=f"adj{i}"))
        for (co, cs) in CH1600:
            for i, (o, s) in enumerate(N2_SPLIT):
                nc.gpsimd.dma_start(adj_sb[i][:, co:co + cs],
                                    adj_d[o:o + s, co:co + cs])

        def load_b(b):
            xp_sb = []
            for k, (o, s) in enumerate(K_BLOCKS_Q):
                t = xp_p.tile([s, N], F32R, name=f"xp{k}", tag=f"xp{k}", bufs=2)
                eng = nc.sync if k % 2 == 0 else nc.scalar
                eng.dma_start(t[:], xp_d[b, o:o + s, :])
                xp_sb.append(t)
            wxt_sb = []
            for i, (o, s) in enumerate(N2_SPLIT):
                t = xpt_p.tile([s, RQ], adt, name=f"wxt{i}", tag=f"wxt{i}", bufs=2)
                eng = nc.sync if i % 2 == 0 else nc.scalar
                eng.dma_start(t[:], wxt_d[b, o:o + s, :])
                wxt_sb.append(t)
            return xp_sb, wxt_sb

        xp0 = load_b(0)

        # remaining (non-critical-path) weights
        wmix1_sb = const.tile([120, 120], adt, name="wmix1")
        nc.sync.dma_start(wmix1_sb[:], wmix1_d[:])
        wmix2_sb = const.tile([120, 120], F32R, name="wmix2")
        nc.sync.dma_start(wmix2_sb[:], wmix2_d[:])
        wskip_sb = []
        KS = [(0, 120), (120, 120), (240, 120), (360, 80)]
        for k, (o, s) in enumerate(KS):
            t = const.tile([s, SQ], F32R, name=f"wskip{k}")
            nc.sync.dma_start(t[:], wskip_d[o:o + s, :])
            wskip_sb.append(t)
        if has_bskip:
            wskipb_sb = const.tile([1, SQ], F32R, name="wskipb")
            nc.sync.dma_start(wskipb_sb[:], wskipb_d[:])
            ones_sb = const.tile([1, N], F32R, name="ones")
            nc.vector.memset(ones_sb[:], 1.0)
        wres_sb = []
        for k, (o, s) in enumerate(K_BLOCKS_Q):
            t = const.tile([s, RQ], F32R, name=f"wres{k}")
            nc.sync.dma_start(t[:], wres_d[o:o + s, :])
            wres_sb.append(t)
        bres_sb = const.tile([120, 1], F32, name="bres_t")
        nc.sync.dma_start(bres_sb[:], bres_d[:])

        def tconv_b(b, xp_sb):
            dres_sb = []
            for m, (mo, ms) in enumerate(M_BLOCKS):
                dr = dres_p.tile([120, N], F32, name=f"dres{m}", tag=f"dres{m}", bufs=1)
                dres_sb.append(dr)
                kts = [m] if m == 3 else [m, m + 1]
                gate_sb = {}
                for gname, bias_sb in (("f", biasf_sb), ("g", biasg_sb)):
                    for (co, cs) in CH800:
                        ps = psA.tile([120, 400], F32, name="tc_ps", tag="psA")
                        for j, kt in enumerate(kts):
                            nc.tensor.matmul(
                                ps[0:ms, :],
                                _mm(wbig_sb[gname][kt][:, mo:mo + ms]),
                                _mm(xp_sb[kt][:, co:co + cs]),
                                start=(j == 0), stop=(j == len(kts) - 1))
                        g = tmp_p.tile([120, 400], F32, name=f"g{gname}",
                                       tag=f"gate{gname}{co}", bufs=2)
                        nc.scalar.activation(
                            g[0:ms, :], ps[0:ms, :],
                            AF.Tanh if gname == "f" else AF.Sigmoid,
                            bias=bias_sb[0:ms, :])
                        gate_sb[(gname, co)] = g
                for (co, cs) in CH800:
                    nc.vector.tensor_mul(dr[0:ms, co:co + cs],
                                         gate_sb[("f", co)][0:ms, :],
                                         gate_sb[("g", co)][0:ms, :])
            return dres_sb

        def hops_b(b, xp_sb, wxt_sb, dres_sb):
            # hop0 + mix1 -> h1T
            h1t_sb = []
            for i, (o, s) in enumerate(N2_SPLIT):
                t = h1t_p.tile([s, RQ], adt, name=f"h1t{i}", tag=f"h1t{i}", bufs=1)
                h1t_sb.append(t)
            h0_tiles = []
            for m, (mo, ms) in enumerate(M_BLOCKS):
                h0 = hop0_p.tile([120, N2], adt, name="h0", tag="h0", bufs=4)
                h0_tiles.append(h0)
                for (co, cs) in CH1600:
                    ps = psA.tile([120, 400], F32, name="h0_ps", tag="psA")
                    nmm = len(N2_SPLIT)
                    for kt in range(nmm):
                        nc.tensor.matmul(
                            ps[0:ms, :],
                            _mm(wxt_sb[kt][:, mo:mo + ms]),
                            _mm(adj_sb[kt][:, co:co + cs]),
                            start=(kt == 0), stop=(kt == nmm - 1))
                    nc.vector.tensor_copy(h0[0:ms, co:co + cs], ps[0:ms, :])
            for m, (mo, ms) in enumerate(M_BLOCKS):
                h0 = h0_tiles[m]
                for i, (o, s) in enumerate(N2_SPLIT):
                    bp = psB.tile([128, 120], F32, name="b1_ps", tag="psB")
                    nc.tensor.matmul(bp[0:s, 0:ms],
                                     _mm(h0[0:ms, o:o + s]),
                                     _mm(wmix1_sb[0:ms, 0:ms]),
                                     start=True, stop=True)
                    nc.vector.tensor_relu(h1t_sb[i][:, mo:mo + ms], bp[0:s, 0:ms])
            # hop1 + mix2 + data_res add -> out_raw
            oraw_sb = []
            h2_tiles = []
            for m, (mo, ms) in enumerate(M_BLOCKS):
                orw = oraw_p.tile([120, N], F32R, name=f"oraw{m}", tag=f"oraw{m}",
                                  bufs=1)
                oraw_sb.append(orw)
                h2 = h2_p.tile([120, N], F32R, name="h2", tag="h2", bufs=4)
                h2_tiles.append(h2)
                for (co, cs) in CH800:
                    ps = psA.tile([120, 400], F32, name="h1_ps", tag="psA")
                    nmm = len(N2_SPLIT)
                    for kt in range(nmm):
                        nc.tensor.matmul(
                            ps[0:ms, :],
                            _mm(h1t_sb[kt][:, mo:mo + ms]),
                            _mm(adj_sb[kt][:, 800 + co:800 + co + cs]),
                            start=(kt == 0), stop=(kt == nmm - 1))
                    nc.scalar.copy(h2[0:ms, co:co + cs], ps[0:ms, :])
            for m, (mo, ms) in enumerate(M_BLOCKS):
                h2 = h2_tiles[m]
                orw = oraw_sb[m]
                for (co, cs) in CH800:
                    ps = psA.tile([120, 400], F32, name="b2_ps", tag="psA")
                    nc.tensor.matmul(ps[0:ms, :],
                                     _mm(wmix2_sb[0:ms, 0:ms]),
                                     _mm(h2[0:ms, co:co + cs]),
                                     start=True, stop=True)
                    rl = tmp_p.tile([120, 400], F32, name="rl", tag=f"rl{co}", bufs=2)
                    nc.scalar.activation(rl[0:ms, :], ps[0:ms, :], AF.Relu)
                    nc.vector.tensor_add(orw[0:ms, co:co + cs], rl[0:ms, :],
                                         dres_sb[m][0:ms, co:co + cs])
            return oraw_sb

        def epilogue_b(b, xp_sb, oraw_sb):
            # skip -> out rows 440:920
            KROWS = [120, 120, 120, 80]
            for sm in range(4):
                for (co, cs) in CH800:
                    ps = psA.tile([120, 400], F32, name="sk_ps", tag="psA")
                    nk = 5 if has_bskip else 4
                    for kt in range(4):
                        nc.tensor.matmul(
                            ps[:, :],
                            _mm(wskip_sb[kt][:, sm * 120:(sm + 1) * 120]),
                            _mm(oraw_sb[kt][0:KROWS[kt], co:co + cs]),
                            start=(kt == 0), stop=(kt == nk - 1))
                    if has_bskip:
                        nc.tensor.matmul(
                            ps[:, :],
                            _mm(wskipb_sb[:, sm * 120:(sm + 1) * 120]),
                            _mm(ones_sb[:, co:co + cs]),
                            start=False, stop=True)
                    sk = fin_p.tile([120, 400], F32, name="sk", tag="sk", bufs=3)
                    nc.scalar.copy(sk[:, :], ps[:, :])
                    nc.scalar.dma_start(
                        out_d[b, RQ + sm * 120:RQ + (sm + 1) * 120, co:co + cs],
                        sk[:, :])
            # residual + final -> out rows 0:440
            for m, (mo, ms) in enumerate(M_BLOCKS):
                for (co, cs) in CH800:
                    ps = psA.tile([120, 400], F32, name="rs_ps", tag="psA")
                    for kt in range(4):
                        nc.tensor.matmul(
                            ps[0:ms, :],
                            _mm(wres_sb[kt][:, mo:mo + ms]),
                            _mm(xp_sb[kt][:, co:co + cs]),
                            start=(kt == 0), stop=(kt == 3))
                    fin = fin_p.tile([120, 400], F32, name="fin", tag="fin", bufs=3)
                    nc.vector.scalar_tensor_tensor(
                        fin[0:ms, :], oraw_sb[m][0:ms, co:co + cs], BN_SCALE,
                        ps[0:ms, :], op0=ALU.mult, op1=ALU.add)
                    if has_bres:
                        nc.vector.tensor_scalar_add(fin[0:ms, :], fin[0:ms, :],
                                                    bres_sb[0:ms, :])
                    nc.scalar.dma_start(out_d[b, mo:mo + ms, co:co + cs],
                                        fin[0:ms, :])

        # software pipeline across batches: epilogue of b-1 is emitted after
        # tconv of b so the PE never drains at batch boundaries
        prev = None
        for b in range(BL):
            xp_sb, wxt_sb = xp0 if b == 0 else load_b(b)
            dres_sb = tconv_b(b, xp_sb)
            if prev is not None:
                epilogue_b(*prev)
            oraw_sb = hops_b(b, xp_sb, wxt_sb, dres_sb)
            prev = (b, xp_sb, oraw_sb)
        epilogue_b(*prev)


def orw_slice(tile_ap, ms, co, cs):
    return tile_ap[0:ms, co:co + cs]


_CACHE = {}


def kernel(**inputs):
    w = _prep_weights(inputs)
    xp_cores, wxt_cores = _prep_data(inputs)

    key = ("prog", w['has_bres'], w['has_bskip'], ADJ_BF16)
    if key not in _CACHE:
        _CACHE[key] = _build_program(has_bres=w['has_bres'],
                                     has_bskip=w['has_bskip'])
    nc = _CACHE[key]

    in_maps = []
    for core in range(NCORES):
        in_maps.append({
            "xp": xp_cores[core],
            "wxt": wxt_cores[core],
            "adj": w['adj'],
            "wbig_f": w['wbig_f'],
            "wbig_g": w['wbig_g'],
            "wmix1": w['wmix1'],
            "wmix2": w['wmix2'],
            "wskip": w['wskip'],
            "wskip_bias": w['wskip_bias'],
            "wres": w['wres'],
            "bias_f": w['bias_f'],
            "bias_g": w['bias_g'],
            "bres": w['bres_tile'],
        })

    import os
    trace = bool(int(os.environ.get("KERNEL_TRACE", "0")))
    res = run_bass_kernel_spmd(nc, in_maps, core_ids=list(range(NCORES)),
                               trace=trace)
    kernel.last_result = res
    outs = [r["out"] for r in res.results]            # each (BL, 920, 800)
    full = np.concatenate(outs, axis=0)               # (32, 920, 800)
    full = full.reshape(B, 23, C, N).transpose(0, 2, 1, 3)   # (B, C, 23, N)
    return np.ascontiguousarray(full)



# revision 2
# speedup vs baseline: 1.3798x; 1.3798x over previous
"""Trainium2 Bass kernel for nn_Net_66975720014255 (gnn_message_passing).

Sharding: data-parallel over batch B=32 across 8 NeuronCores (4 batches per
core); adjacency and all weights replicated. No collectives.

Precision strategy (rel-err gate 2e-2; achieves ~5e-3):
  - hop0 (h @ adj, K=1600) and hop1 run as fp8e4m3 DoubleRow matmuls: one
    PE pass covers two 128-row K-tiles -> 2x throughput.
  - The dominant fp8 error is quantization of the data windows h; that
    error lives in adj's top singular subspace and is re-amplified by
    hop1. Since eps = h - fp8(h) is known on the host, we ship
    P = S_x*(Ur^T eps) and Cr = S_a*(Ur^T adj) (rank-64) and add one
    K=64 fp16 matmul per hop0 PSUM group, cancelling that component.
  - tconv + residual also run fp8 (error-neutral); skip runs fp16 (fp8
    too lossy there); channel mixes run bf16 with all fp8 scales and the
    BatchNorm constant folded into the mix weights.

Per-core device program (C=40, T=12, N=800, R=11, 4 local batches):
  tconv -> dres'; hop0(DR fp8)+corr -> h0(bf16); mix1 -> h1t(fp8, paired,
  transposed); hop1(DR fp8) -> h2(bf16); mix2+relu -> oraw'(fp16);
  skip = wskip^T oraw' (fp16); fin = oraw' + desc*res_psum (res fp8 DR).
Batches are software-pipelined: epilogue of batch b-1 is emitted after
tconv of batch b so the PE never drains.
"""

import math
import sys

if '/opt/trn_rl_repo' not in sys.path:
    sys.path.insert(0, '/opt/trn_rl_repo')

import numpy as np
import ml_dtypes

import concourse.bass as bass  # noqa: F401
import concourse.tile as tile
from concourse import bacc, mybir
from concourse.bass_utils import run_bass_kernel_spmd

# ----- problem constants (hardcoded per contract) -----
B, C, T, N = 32, 40, 12, 800
R = T - 1                    # 11
N2 = 2 * N                   # 1600
NCORES = 8
BL = B // NCORES             # 4 local batches per core
BN_SCALE = float(1.0 / np.sqrt(1.0 + 1e-5))

Q = T * C                    # 480 rows (t,c) per batch
RQ = R * C                   # 440 rows (r,c) per batch
SQ = 12 * C                  # 480 skip rows (s,c) per batch

M_BLOCKS = [(0, 120), (120, 120), (240, 120), (360, 80)]     # (r,c) row blocks
KS_SKIP = [(0, 120), (120, 120), (240, 120), (360, 80)]      # oraw K tiles
N2_SPLIT = [(k * 128, 128) for k in range(12)] + [(1536, 64)]
CH800 = [(0, 400), (400, 400)]
CH1600 = [(0, 400), (400, 400), (800, 400), (1200, 400)]
NPAIR = 6                    # six 256-row DR pairs + 64 tail
MPAD = 448                   # lhsT pair-tile cols: 440 padded to 16B-aligned

# fp8 scales (powers of two; fp8 relative error is scale-free, margins wide)
S_X = 32.0                   # data |x| <~7 -> <=224
S_A = 256.0                  # adj max ~0.4 -> ~102
S_H1 = 8.0                   # h1 max ~10 -> ~80
S_W = 256.0                  # conv/res weight entries ~0.45 -> ~115
RCORR = 64                   # correction rank
F8MAX = 240.0

F32 = mybir.dt.float32
F16 = mybir.dt.float16
BF16 = mybir.dt.bfloat16
F8 = mybir.dt.float8e4
np_f8 = ml_dtypes.float8_e4m3
np_bf16 = ml_dtypes.bfloat16


def _q8(a, scale):
    """Saturating fp8e4m3 quantization of a*scale."""
    return np.clip(np.asarray(a, np.float32) * scale, -F8MAX, F8MAX).astype(np_f8)


def _pack_pairs(m, rows_per_tile, npair, mpad):
    """[K, M] -> [npair, rows, 2, mpad] fp8 pair tiles for DoubleRow."""
    out = np.zeros((npair, rows_per_tile, 2, mpad), np_f8)
    step = 2 * rows_per_tile
    for j in range(npair):
        out[j, :, 0, :m.shape[1]] = m[step * j: step * j + rows_per_tile]
        out[j, :, 1, :m.shape[1]] = m[step * j + rows_per_tile: step * (j + 1)]
    return out


# ---------------------------------------------------------------------------
# host-side preparation (pure numpy)
# ---------------------------------------------------------------------------

def _prep(inp):
    f32 = np.float32
    nv1, nv2 = np.asarray(inp['nv1'], f32), np.asarray(inp['nv2'], f32)
    adj = np.maximum(f32(0), nv1 @ nv2)                       # (1600,1600)

    x = np.asarray(inp['x'], f32) + np.asarray(inp['t_emb'], f32) \
        + np.asarray(inp['s_emb'], f32)                       # (B,C,T,N)
    xp = np.ascontiguousarray(x.transpose(0, 2, 1, 3)).reshape(B, Q, N)
    xpt = np.ascontiguousarray(x.transpose(0, 3, 2, 1)).reshape(B, N, Q)
    wxt = np.concatenate([xpt[:, :, :RQ], xpt[:, :, C:]], axis=1)  # (B,1600,440)

    # fp8 data + paired layouts
    adj8 = _q8(adj, S_A)
    adjp = np.zeros((NPAIR, 128, 2, N2), np_f8)
    for j in range(NPAIR):
        adjp[j, :, 0] = adj8[256 * j: 256 * j + 128]
        adjp[j, :, 1] = adj8[256 * j + 128: 256 * j + 256]
    adjt = np.ascontiguousarray(adj8[1536:1600])              # (64,1600)

    wxt8 = _q8(wxt, S_X)                                      # (B,1600,440)
    wxtp = np.zeros((B, NPAIR, 128, 2, MPAD), np_f8)
    for j in range(NPAIR):
        wxtp[:, j, :, 0, :RQ] = wxt8[:, 256 * j: 256 * j + 128]
        wxtp[:, j, :, 1, :RQ] = wxt8[:, 256 * j + 128: 256 * j + 256]
    wxtt = np.ascontiguousarray(wxt8[:, 1536:1600])           # (B,64,440)

    xp8 = _q8(xp, S_X)                                        # (B,480,800)
    xp8p = np.zeros((B, 2, 120, 2, N), np_f8)
    for p in range(2):
        xp8p[:, p, :, 0] = xp8[:, 240 * p: 240 * p + 120]
        xp8p[:, p, :, 1] = xp8[:, 240 * p + 120: 240 * p + 240]

    # rank-RCORR left singular basis of adj (randomized subspace iteration)
    rng = np.random.default_rng(0)
    G = rng.standard_normal((N2, RCORR + 16)).astype(f32)
    Y = adj @ (adj.T @ (adj @ G))
    Qb, _ = np.linalg.qr(Y)
    Ur = np.ascontiguousarray(Qb[:, :RCORR])                  # (1600, 64)
    eps = wxt - wxt8.astype(f32) / f32(S_X)                   # (B,1600,440)
    pt = np.einsum('kr,bkm->brm', Ur, eps, optimize=True)     # (B,64,440)
    pt = (pt * f32(S_X)).astype(np.float16)
    cr = (Ur.T @ adj * f32(S_A)).astype(np.float16)           # (64,1600)

    def wbig(W):
        Wb = np.zeros((Q, RQ), f32)
        W0, W1 = np.asarray(W[:, :, 0], f32), np.asarray(W[:, :, 1], f32)
        for r in range(R):
            Wb[r * C:(r + 1) * C, r * C:(r + 1) * C] = W0.T
            Wb[(r + 1) * C:(r + 2) * C, r * C:(r + 1) * C] = W1.T
        return Wb

    wbigpf = _pack_pairs(_q8(wbig(inp['W_f']), S_W), 120, 2, MPAD)
    wbigpg = _pack_pairs(_q8(wbig(inp['W_g']), S_W), 120, 2, MPAD)

    wres = np.zeros((Q, RQ), f32)
    eye = np.eye(C, dtype=f32)
    Wr = np.asarray(inp['W_res'], f32) * BN_SCALE             # (11,12)
    for t in range(T):
        for r in range(R):
            wres[t * C:(t + 1) * C, r * C:(r + 1) * C] = Wr[r, t] * eye
    wresp = _pack_pairs(_q8(wres, S_W), 120, 2, MPAD)

    # skip weights fp16, no BN (rhs oraw' already carries BN)
    wskip = np.zeros((RQ, SQ), f32)
    Ws = np.asarray(inp['W_skip'], f32)                       # (12,11)
    bs = np.asarray(inp['b_skip'], f32) * BN_SCALE
    for s in range(12):
        for r in range(R):
            wskip[r * C:(r + 1) * C, s * C:(s + 1) * C] = Ws[s, r] * eye
    wskip16 = wskip.astype(np.float16)
    wskip_bias = np.zeros((1, SQ), np.float16)
    for s in range(12):
        wskip_bias[0, s * C:(s + 1) * C] = bs[s]

    def blkdiag3(A):
        M_ = np.zeros((120, 120), f32)
        for j in range(3):
            M_[j * C:(j + 1) * C, j * C:(j + 1) * C] = A
        return M_

    wmix1 = (blkdiag3(np.asarray(inp['W_gcn'][0], f32).T)
             * f32(S_H1 / (S_X * S_A))).astype(np_bf16)
    wmix2 = (blkdiag3(np.asarray(inp['W_gcn'][1], f32).T)
             * f32(BN_SCALE / (S_H1 * S_A))).astype(np_bf16)

    bias_f = np.ascontiguousarray(np.tile(np.asarray(inp['b_f'], f32), 3)[:, None])
    bias_g = np.ascontiguousarray(np.tile(np.asarray(inp['b_g'], f32), 3)[:, None])

    bres = np.asarray(inp['b_res'], f32) * BN_SCALE
    bres_tile = np.zeros((120, 1), f32)
    for p in range(120):
        r = p // C
        bres_tile[p, 0] = bres[r] if r < R else 0.0

    shared = dict(adjp=adjp, adjt=adjt, cr=cr,
                  wbigpf=wbigpf.reshape(2 * 120, 2, MPAD),
                  wbigpg=wbigpg.reshape(2 * 120, 2, MPAD),
                  wresp=wresp.reshape(2 * 120, 2, MPAD),
                  wskip=wskip16, wskip_bias=wskip_bias,
                  wmix1=wmix1, wmix2=wmix2,
                  bias_f=bias_f, bias_g=bias_g, bres=bres_tile)
    per_core = []
    for c_ in range(NCORES):
        sl = slice(c_ * BL, (c_ + 1) * BL)
        ptc = np.zeros((RCORR, BL * RQ), np.float16)
        for b in range(BL):
            ptc[:, b * RQ:(b + 1) * RQ] = pt[c_ * BL + b]
        per_core.append(dict(
            wxtp=np.ascontiguousarray(wxtp[sl]).reshape(BL * NPAIR, 128, 2, MPAD),
            wxtt=np.ascontiguousarray(wxtt[sl]),
            xp8p=np.ascontiguousarray(xp8p[sl]).reshape(BL * 2, 120, 2, N),
            pt=ptc))
    has_bres = bool(np.any(bres))
    has_bskip = bool(np.any(bs))
    return shared, per_core, has_bres, has_bskip


# ---------------------------------------------------------------------------
# device program
# ---------------------------------------------------------------------------

def _build_program(has_bres, has_bskip):
    nc = bacc.Bacc("TRN2", target_bir_lowering=False, debug=False,
                   enable_asserts=False, num_devices=NCORES)

    wxtp_d = nc.dram_tensor("wxtp", [BL * NPAIR, 128, 2, MPAD], F8,
                            kind="ExternalInput").ap()
    wxtt_d = nc.dram_tensor("wxtt", [BL, 64, RQ], F8, kind="ExternalInput").ap()
    xp8p_d = nc.dram_tensor("xp8p", [BL * 2, 120, 2, N], F8,
                            kind="ExternalInput").ap()
    pt_d = nc.dram_tensor("pt", [RCORR, BL * RQ], F16, kind="ExternalInput").ap()
    adjp_d = nc.dram_tensor("adjp", [NPAIR, 128, 2, N2], F8,
                            kind="ExternalInput").ap()
    adjt_d = nc.dram_tensor("adjt", [64, N2], F8, kind="ExternalInput").ap()
    cr_d = nc.dram_tensor("cr", [RCORR, N2], F16, kind="ExternalInput").ap()
    wbigpf_d = nc.dram_tensor("wbigpf", [240, 2, MPAD], F8,
                              kind="ExternalInput").ap()
    wbigpg_d = nc.dram_tensor("wbigpg", [240, 2, MPAD], F8,
                              kind="ExternalInput").ap()
    wresp_d = nc.dram_tensor("wresp", [240, 2, MPAD], F8,
                             kind="ExternalInput").ap()
    wskip_d = nc.dram_tensor("wskip", [RQ, SQ], F16, kind="ExternalInput").ap()
    wskipb_d = nc.dram_tensor("wskip_bias", [1, SQ], F16,
                              kind="ExternalInput").ap()
    wmix1_d = nc.dram_tensor("wmix1", [120, 120], BF16, kind="ExternalInput").ap()
    wmix2_d = nc.dram_tensor("wmix2", [120, 120], BF16, kind="ExternalInput").ap()
    biasf_d = nc.dram_tensor("bias_f", [120, 1], F32, kind="ExternalInput").ap()
    biasg_d = nc.dram_tensor("bias_g", [120, 1], F32, kind="ExternalInput").ap()
    bres_d = nc.dram_tensor("bres", [120, 1], F32, kind="ExternalInput").ap()
    # output rows per batch: 0:440 final (r,c), 440:920 skip (s,c)
    out_d = nc.dram_tensor("out", [BL, 920, N], F32, kind="ExternalOutput").ap()

    with tile.TileContext(nc) as tc:
        _emit(nc, tc, dict(wxtp=wxtp_d, wxtt=wxtt_d, xp8p=xp8p_d, pt=pt_d,
                           adjp=adjp_d, adjt=adjt_d, cr=cr_d, wbigpf=wbigpf_d,
                           wbigpg=wbigpg_d, wresp=wresp_d, wskip=wskip_d,
                           wskipb=wskipb_d, wmix1=wmix1_d, wmix2=wmix2_d,
                           biasf=biasf_d, biasg=biasg_d, bres=bres_d,
                           out=out_d),
              has_bres, has_bskip)
    nc.compile()
    return nc


def _emit(nc, tc, d, has_bres, has_bskip):
    from contextlib import ExitStack
    AF = mybir.ActivationFunctionType
    ALU = mybir.AluOpType
    DR = mybir.MatmulPerfMode.DoubleRow
    DESC_G = float(1.0 / (S_X * S_W))          # gate pre-activation descale
    DESC_R = float(1.0 / (S_X * S_W))          # res psum descale (BN in wres)

    ctx = ExitStack()
    with ctx:
        const = ctx.enter_context(tc.tile_pool(name="const", bufs=1))
        xp8_p = ctx.enter_context(tc.tile_pool(name="xp8", bufs=2))
        wxt_p = ctx.enter_context(tc.tile_pool(name="wxt", bufs=2))
        dres_p = ctx.enter_context(tc.tile_pool(name="dres", bufs=2))
        h0_p = ctx.enter_context(tc.tile_pool(name="h0", bufs=2))
        h1t_p = ctx.enter_context(tc.tile_pool(name="h1t", bufs=2))
        h2_p = ctx.enter_context(tc.tile_pool(name="h2", bufs=2))
        oraw_p = ctx.enter_context(tc.tile_pool(name="oraw", bufs=2))
        tmp_p = ctx.enter_context(tc.tile_pool(name="tmp", bufs=2))
        fin_p = ctx.enter_context(tc.tile_pool(name="fin", bufs=4))
        psA = ctx.enter_context(tc.tile_pool(name="psA", bufs=6, space="PSUM"))
        psB = ctx.enter_context(tc.tile_pool(name="psB", bufs=2, space="PSUM"))

        # ---- DMA: tconv-critical first ----
        biasf_sb = const.tile([120, 1], F32, name="biasf")
        nc.sync.dma_start(biasf_sb[:], d['biasf'][:])
        biasg_sb = const.tile([120, 1], F32, name="biasg")
        nc.scalar.dma_start(biasg_sb[:], d['biasg'][:])
        wbig_sb = {}
        for gname, wd in (("f", d['wbigpf']), ("g", d['wbigpg'])):
            tiles = []
            for p in range(2):
                t = const.tile([120, 2, MPAD], F8, name=f"wbigp{gname}{p}")
                eng = nc.sync if gname == "f" else nc.scalar
                eng.dma_start(t[:, :, :], wd[120 * p:120 * (p + 1), :, :])
                tiles.append(t)
            wbig_sb[gname] = tiles

        def load_xp8(b):
            xp8_sb = []
            for p in range(2):
                t = xp8_p.tile([120, 2, N], F8, name=f"xp8p{p}",
                               tag=f"xp8p{p}", bufs=2)
                eng = nc.sync if p == 0 else nc.scalar
                eng.dma_start(t[:, :, :], d['xp8p'][b * 2 + p, :, :, :])
                xp8_sb.append(t)
            return xp8_sb

        def load_wxt(b):
            wxtp_sb = []
            for j in range(NPAIR):
                t = wxt_p.tile([128, 2, MPAD], F8, name=f"wxtp{j}",
                               tag=f"wxtp{j}", bufs=2)
                eng = nc.sync if j % 2 == 0 else nc.scalar
                eng.dma_start(t[:, :, :], d['wxtp'][b * NPAIR + j, :, :, :])
                wxtp_sb.append(t)
            tt = wxt_p.tile([64, RQ], F8, name="wxtt", tag="wxtt", bufs=2)
            nc.sync.dma_start(tt[:, :], d['wxtt'][b, :, :])
            return wxtp_sb, tt

        xp8_0 = load_xp8(0)

        # correction operands + adjacency; adj in column-chunk-major order so
        # hop0's first chunk starts after ~1/4 of the adj bytes
        pt_sb = const.tile([RCORR, BL * RQ], F16, name="pt")
        nc.sync.dma_start(pt_sb[:], d['pt'][:])
        cr_sb = const.tile([RCORR, N2], F16, name="cr")
        nc.sync.dma_start(cr_sb[:], d['cr'][:])
        adjp_sb = [const.tile([128, 2, N2], F8, name=f"adjp{j}")
                   for j in range(NPAIR)]
        adjt_sb = const.tile([64, N2], F8, name="adjt")
        for (co, cs) in CH1600:
            for j in range(NPAIR):
                nc.gpsimd.dma_start(adjp_sb[j][:, :, co:co + cs],
                                    d['adjp'][j, :, :, co:co + cs])
            nc.gpsimd.dma_start(adjt_sb[:, co:co + cs],
                                d['adjt'][:, co:co + cs])

        wxt_0 = load_wxt(0)

        wmix1_sb = const.tile([120, 120], BF16, name="wmix1")
        nc.sync.dma_start(wmix1_sb[:], d['wmix1'][:])
        wmix2_sb = const.tile([120, 120], BF16, name="wmix2")
        nc.sync.dma_start(wmix2_sb[:], d['wmix2'][:])
        wskip_sb = []
        for k, (o, s) in enumerate(KS_SKIP):
            t = const.tile([s, SQ], F16, name=f"wskip{k}")
            nc.sync.dma_start(t[:], d['wskip'][o:o + s, :])
            wskip_sb.append(t)
        if has_bskip:
            wskipb_sb = const.tile([1, SQ], F16, name="wskipb")
            nc.sync.dma_start(wskipb_sb[:], d['wskipb'][:])
            ones_sb = const.tile([1, N], F16, name="ones")
            nc.vector.memset(ones_sb[:], 1.0)
        wresp_sb = []
        for p in range(2):
            t = const.tile([120, 2, MPAD], F8, name=f"wresp{p}")
            nc.sync.dma_start(t[:, :, :], d['wresp'][120 * p:120 * (p + 1), :, :])
            wresp_sb.append(t)
        bres_sb = const.tile([120, 1], F32, name="bres_t")
        nc.sync.dma_start(bres_sb[:], d['bres'][:])

        def tconv_b(b, xp8_sb):
            dres_sb = []
            for m, (mo, ms) in enumerate(M_BLOCKS):
                dr_t = dres_p.tile([120, N], F16, name=f"dres{m}",
                                   tag=f"dres{m}", bufs=2)
                dres_sb.append(dr_t)
                gate_sb = {}
                for gname, bias_sb in (("f", biasf_sb), ("g", biasg_sb)):
                    wp = wbig_sb[gname]
                    for (co, cs) in CH800:
                        ps = psA.tile([120, 400], F32, name="tc_ps", tag="psA")
                        if m == 0 or m == 2:
                            nc.tensor.matmul(
                                ps[0:ms, :], wp[m // 2][:, :, mo:mo + ms],
                                xp8_sb[m // 2][:, :, co:co + cs],
                                start=True, stop=True, perf_mode=DR)
                        elif m == 1:
                            nc.tensor.matmul(
                                ps[0:ms, :], wp[0][:, 1, mo:mo + ms],
                                xp8_sb[0][:, 1, co:co + cs],
                                start=True, stop=False)
                            nc.tensor.matmul(
                                ps[0:ms, :], wp[1][:, 0, mo:mo + ms],
                                xp8_sb[1][:, 0, co:co + cs],
                                start=False, stop=True)
                        else:
                            nc.tensor.matmul(
                                ps[0:ms, :], wp[1][:, 1, mo:mo + ms],
                                xp8_sb[1][:, 1, co:co + cs],
                                start=True, stop=True)
                        g = tmp_p.tile([120, 400], F16, name=f"g{gname}",
                                       tag=f"gate{gname}{co}", bufs=2)
                        nc.scalar.activation(
                            g[0:ms, :], ps[0:ms, :],
                            AF.Tanh if gname == "f" else AF.Sigmoid,
                            bias=bias_sb[0:ms, :], scale=DESC_G)
                        gate_sb[(gname, co)] = g
                for (co, cs) in CH800:
                    # dres' = BN * tanh * sigmoid
                    nc.vector.scalar_tensor_tensor(
                        dr_t[0:ms, co:co + cs], gate_sb[("f", co)][0:ms, :],
                        BN_SCALE, gate_sb[("g", co)][0:ms, :],
                        op0=ALU.mult, op1=ALU.mult)
            return dres_sb

        def hops_b(b, wxtp_sb, wxtt_sb, dres_sb):
            # hop0 + rank correction -> h0 (bf16, scaled by S_X*S_A)
            h0_sb = []
            for m, (mo, ms) in enumerate(M_BLOCKS):
                h0 = h0_p.tile([120, N2], BF16, name=f"h0_{m}", tag=f"h0_{m}",
                               bufs=2)
                h0_sb.append(h0)
                for (co, cs) in CH1600:
                    ps = psA.tile([120, 400], F32, name="h0_ps", tag="psA")
                    for j in range(NPAIR):
                        nc.tensor.matmul(ps[0:ms, :],
                                         wxtp_sb[j][:, :, mo:mo + ms],
                                         adjp_sb[j][:, :, co:co + cs],
                                         start=(j == 0), stop=False,
                                         perf_mode=DR)
                    nc.tensor.matmul(ps[0:ms, :], wxtt_sb[:, mo:mo + ms],
                                     adjt_sb[:, co:co + cs],
                                     start=False, stop=False)
                    nc.tensor.matmul(ps[0:ms, :],
                                     pt_sb[:, b * RQ + mo: b * RQ + mo + ms],
                                     cr_sb[:, co:co + cs],
                                     start=False, stop=True)
                    nc.vector.tensor_copy(h0[0:ms, co:co + cs], ps[0:ms, :])
            # mix1 -> h1t fp8 paired (nodes on partitions), scaled by S_H1
            h1tp_sb = [h1t_p.tile([128, 2, MPAD], F8, name=f"h1tp{j}",
                                  tag=f"h1tp{j}", bufs=2) for j in range(NPAIR)]
            h1tt_sb = h1t_p.tile([64, RQ], F8, name="h1tt", tag="h1tt", bufs=2)
            for m, (mo, ms) in enumerate(M_BLOCKS):
                for i, (o, s) in enumerate(N2_SPLIT):
                    bp = psB.tile([128, 120], F32, name="m1_ps", tag="psB")
                    nc.tensor.matmul(bp[0:s, 0:ms], h0_sb[m][0:ms, o:o + s],
                                     wmix1_sb[0:ms, 0:ms],
                                     start=True, stop=True)
                    if i < 12:
                        nc.vector.tensor_relu(
                            h1tp_sb[i // 2][0:s, i % 2, mo:mo + ms],
                            bp[0:s, 0:ms])
                    else:
                        nc.vector.tensor_relu(h1tt_sb[0:s, mo:mo + ms],
                                              bp[0:s, 0:ms])
            # hop1 -> h2 (bf16, scaled by S_H1*S_A)
            h2_sb = []
            for m, (mo, ms) in enumerate(M_BLOCKS):
                h2 = h2_p.tile([120, N], BF16, name=f"h2_{m}", tag=f"h2_{m}",
                               bufs=2)
                h2_sb.append(h2)
                for (co, cs) in CH800:
                    ps = psA.tile([120, 400], F32, name="h1_ps", tag="psA")
                    for j in range(NPAIR):
                        nc.tensor.matmul(ps[0:ms, :],
                                         h1tp_sb[j][:, :, mo:mo + ms],
                                         adjp_sb[j][:, :, 800 + co:800 + co + cs],
                                         start=(j == 0), stop=False,
                                         perf_mode=DR)
                    nc.tensor.matmul(ps[0:ms, :], h1tt_sb[:, mo:mo + ms],
                                     adjt_sb[:, 800 + co:800 + co + cs],
                                     start=False, stop=True)
                    nc.scalar.copy(h2[0:ms, co:co + cs], ps[0:ms, :])
            # mix2 + relu (+BN via wmix2) + dres -> oraw' (fp16)
            oraw_sb = []
            for m, (mo, ms) in enumerate(M_BLOCKS):
                orw = oraw_p.tile([120, N], F16, name=f"oraw{m}", tag=f"oraw{m}",
                                  bufs=2)
                oraw_sb.append(orw)
                for (co, cs) in CH800:
                    ps = psA.tile([120, 400], F32, name="m2_ps", tag="psA")
                    nc.tensor.matmul(ps[0:ms, :], wmix2_sb[0:ms, 0:ms],
                                     h2_sb[m][0:ms, co:co + cs],
                                     start=True, stop=True)
                    rl = tmp_p.tile([120, 400], F16, name="rl", tag=f"rl{co}",
                                    bufs=2)
                    nc.scalar.activation(rl[0:ms, :], ps[0:ms, :], AF.Relu)
                    nc.vector.tensor_add(orw[0:ms, co:co + cs], rl[0:ms, :],
                                         dres_sb[m][0:ms, co:co + cs])
            return oraw_sb

        def epilogue_b(b, xp8_sb, oraw_sb):
            # skip -> out rows 440:920  (skip = wskip^T @ oraw', fp16)
            for sm in range(4):
                for (co, cs) in CH800:
                    ps = psA.tile([120, 400], F32, name="sk_ps", tag="psA")
                    for kt, (o, s) in enumerate(KS_SKIP):
                        nc.tensor.matmul(
                            ps[:, :],
                            wskip_sb[kt][:, sm * 120:(sm + 1) * 120],
                            oraw_sb[kt][0:s, co:co + cs],
                            start=(kt == 0),
                            stop=(kt == 3 and not has_bskip))
                    if has_bskip:
                        nc.tensor.matmul(ps[:, :],
                                         wskipb_sb[:, sm * 120:(sm + 1) * 120],
                                         ones_sb[:, co:co + cs],
                                         start=False, stop=True)
                    sk = fin_p.tile([120, 400], F32, name="sk", tag="sk", bufs=3)
                    nc.scalar.copy(sk[:, :], ps[:, :])
                    nc.scalar.dma_start(
                        d['out'][b, RQ + sm * 120:RQ + (sm + 1) * 120,
                                 co:co + cs], sk[:, :])
            # residual (fp8 DR) + final -> out rows 0:440
            for m, (mo, ms) in enumerate(M_BLOCKS):
                for (co, cs) in CH800:
                    ps = psA.tile([120, 400], F32, name="rs_ps", tag="psA")
                    for p in range(2):
                        nc.tensor.matmul(ps[0:ms, :],
                                         wresp_sb[p][:, :, mo:mo + ms],
                                         xp8_sb[p][:, :, co:co + cs],
                                         start=(p == 0), stop=(p == 1),
                                         perf_mode=DR)
                    fin = fin_p.tile([120, 400], F32, name="fin", tag="fin",
                                     bufs=3)
                    nc.vector.scalar_tensor_tensor(
                        fin[0:ms, :], ps[0:ms, :], DESC_R,
                        oraw_sb[m][0:ms, co:co + cs],
                        op0=ALU.mult, op1=ALU.add)
                    if has_bres:
                        nc.vector.tensor_scalar_add(fin[0:ms, :], fin[0:ms, :],
                                                    bres_sb[0:ms, :])
                    nc.scalar.dma_start(d['out'][b, mo:mo + ms, co:co + cs],
                                        fin[0:ms, :])

        # software pipeline across batches
        prev = None
        for b in range(BL):
            xp8_sb = xp8_0 if b == 0 else load_xp8(b)
            wxtp_sb, wxtt_sb = wxt_0 if b == 0 else load_wxt(b)
            dres_sb = tconv_b(b, xp8_sb)
            if prev is not None:
                epilogue_b(*prev)
            oraw_sb = hops_b(b, wxtp_sb, wxtt_sb, dres_sb)
            prev = (b, xp8_sb, oraw_sb)
        epilogue_b(*prev)


_CACHE = {}


def kernel(**inputs):
    shared, per_core, has_bres, has_bskip = _prep(inputs)

    key = ("prog", has_bres, has_bskip)
    if key not in _CACHE:
        _CACHE[key] = _build_program(has_bres, has_bskip)
    nc = _CACHE[key]

    in_maps = []
    for core in range(NCORES):
        m = dict(shared)
        m.update(per_core[core])
        in_maps.append(m)

    import os
    trace = bool(int(os.environ.get("KERNEL_TRACE", "0")))
    res = run_bass_kernel_spmd(nc, in_maps, core_ids=list(range(NCORES)),
                               trace=trace)
    kernel.last_result = res
    outs = [r["out"] for r in res.results]            # each (BL, 920, 800)
    full = np.concatenate(outs, axis=0)               # (32, 920, 800)
    full = full.reshape(B, 23, C, N).transpose(0, 2, 1, 3)   # (B, C, 23, N)
    return np.ascontiguousarray(full)


# revision 7
# speedup vs baseline: 1.5338x; 1.1116x over previous
"""Trainium2 Bass kernel for nn_Net_66975720014255 (gnn_message_passing).

Sharding: data-parallel over batch B=32 across 8 NeuronCores (4 batches per
core); adjacency and all weights replicated. No collectives.

Precision strategy (rel-err gate 2e-2; achieves ~5e-3):
  - hop0 (h @ adj, K=1600) and hop1 run as fp8e4m3 DoubleRow matmuls: one
    PE pass covers two 128-row K-tiles -> 2x throughput.
  - The dominant fp8 error is quantization of the data windows h; that
    error lives in adj's top singular subspace and is re-amplified by
    hop1. Since eps = h - fp8(h) is known on the host, we ship
    P = S_x*(Ur^T eps) and Cr = S_a*(Ur^T adj) (rank-64) and add one
    fp16 matmul per hop0 PSUM group, cancelling that component. The
    64-row adjacency K-tail rides in the same matmul (K=64+64) in fp16.
  - tconv + residual also run fp8 (error-neutral); skip runs fp16 (fp8
    too lossy there); channel mixes run bf16 with all fp8 scales and the
    BatchNorm constant folded into the mix weights. Output DMA is fp16.

Per-core device program (C=40, T=12, N=800, R=11, 4 local batches):
  tconv -> dres'; hop0(DR fp8)+corr -> h0(bf16); mix1 -> h1t(fp8, paired,
  transposed); hop1(DR fp8) -> h2(bf16); mix2+relu -> oraw'(fp16);
  skip = wskip^T oraw' (fp16); fin = oraw' + desc*res_psum (res fp8 DR).
Pipeline: tconv(0), tconv(1) run first (covers the adjacency DMA), then
hops(b) / epilogue(b-1) / tconv(b+2) interleave so no engine drains.
"""

import sys

if '/opt/trn_rl_repo' not in sys.path:
    sys.path.insert(0, '/opt/trn_rl_repo')

import numpy as np
import ml_dtypes

import concourse.bass as bass  # noqa: F401
import concourse.tile as tile
from concourse import bacc, mybir
from concourse.bass_utils import run_bass_kernel_spmd

# ----- problem constants (hardcoded per contract) -----
B, C, T, N = 32, 40, 12, 800
R = T - 1                    # 11
N2 = 2 * N                   # 1600
NCORES = 8
BL = B // NCORES             # 4 local batches per core
BN_SCALE = float(1.0 / np.sqrt(1.0 + 1e-5))

Q = T * C                    # 480 rows (t,c) per batch
RQ = R * C                   # 440 rows (r,c) per batch
SQ = 12 * C                  # 480 skip rows (s,c) per batch

M_BLOCKS = [(0, 120), (120, 120), (240, 120), (360, 80)]     # (r,c) row blocks
KS_SKIP = [(0, 120), (120, 120), (240, 120), (360, 80)]      # oraw K tiles
N2_SPLIT = [(k * 128, 128) for k in range(12)] + [(1536, 64)]
CH800 = [(0, 400), (400, 400)]
CH1600 = [(0, 400), (400, 400), (800, 400), (1200, 400)]
NPAIR = 6                    # six 256-row DR pairs; 64-row tail via fp16 corr
MPAD = 448                   # lhsT pair-tile cols: 440 padded to 16B-aligned

# fp8 scales (powers of two; fp8 relative error is scale-free, margins wide)
S_X = 32.0                   # data |x| <~7 -> <=224
S_A = 256.0                  # adj max ~0.4 -> ~102
S_H1 = 8.0                   # h1 max ~10 -> ~80
S_W = 256.0                  # conv/res weight entries ~0.45 -> ~115
RCORR = 64                   # correction rank
KCORR = RCORR + 64           # + adjacency 64-row tail in fp16
F8MAX = 240.0

F32 = mybir.dt.float32
F16 = mybir.dt.float16
BF16 = mybir.dt.bfloat16
F8 = mybir.dt.float8e4
np_f8 = ml_dtypes.float8_e4m3
np_bf16 = ml_dtypes.bfloat16


def _q8(a, scale):
    """Saturating fp8e4m3 quantization of a*scale."""
    return np.clip(np.asarray(a, np.float32) * scale, -F8MAX, F8MAX).astype(np_f8)


def _pack_pairs(m, rows, npair, mpad, offs=None):
    """[K, M] -> [npair, rows, 2, mpad] fp8 pair tiles for DoubleRow.

    offs[j] = starting row of pair j (halves at offs[j], offs[j]+rows)."""
    out = np.zeros((npair, rows, 2, mpad), np_f8)
    for j in range(npair):
        o = offs[j] if offs is not None else 2 * rows * j
        out[j, :, 0, :m.shape[1]] = m[o: o + rows]
        out[j, :, 1, :m.shape[1]] = m[o + rows: o + 2 * rows]
    return out


# ---------------------------------------------------------------------------
# host-side preparation (pure numpy)
# ---------------------------------------------------------------------------

def _prep(inp):
    f32 = np.float32
    nv1, nv2 = np.asarray(inp['nv1'], f32), np.asarray(inp['nv2'], f32)
    adj = np.maximum(f32(0), nv1 @ nv2)                       # (1600,1600)

    x = np.asarray(inp['x'], f32) + np.asarray(inp['t_emb'], f32) \
        + np.asarray(inp['s_emb'], f32)                       # (B,C,T,N)
    xp = np.ascontiguousarray(x.transpose(0, 2, 1, 3)).reshape(B, Q, N)
    xpt = np.ascontiguousarray(x.transpose(0, 3, 2, 1)).reshape(B, N, Q)
    wxt = np.concatenate([xpt[:, :, :RQ], xpt[:, :, C:]], axis=1)  # (B,1600,440)

    # fp8 data + paired layouts
    adj8 = _q8(adj, S_A)
    adjp = np.zeros((NPAIR, 128, 2, N2), np_f8)
    for j in range(NPAIR):
        adjp[j, :, 0] = adj8[256 * j: 256 * j + 128]
        adjp[j, :, 1] = adj8[256 * j + 128: 256 * j + 256]
    adjt = np.ascontiguousarray(adj8[1536:1600])              # (64,1600) hop1 tail

    wxt8 = _q8(wxt, S_X)                                      # (B,1600,440)
    wxtp = np.zeros((B, NPAIR, 128, 2, MPAD), np_f8)
    for j in range(NPAIR):
        wxtp[:, j, :, 0, :RQ] = wxt8[:, 256 * j: 256 * j + 128]
        wxtp[:, j, :, 1, :RQ] = wxt8[:, 256 * j + 128: 256 * j + 256]

    xp8 = _q8(xp, S_X)                                        # (B,480,800)
    # even pairs (kt0,kt1),(kt2,kt3) + odd pair (kt1,kt2)
    xp8p = np.zeros((B, 3, 120, 2, N), np_f8)
    for p, o in enumerate((0, 240, 120)):
        xp8p[:, p, :, 0] = xp8[:, o: o + 120]
        xp8p[:, p, :, 1] = xp8[:, o + 120: o + 240]

    # rank-RCORR left singular basis of adj (randomized subspace iteration)
    rng = np.random.default_rng(0)
    G = rng.standard_normal((N2, RCORR + 16)).astype(f32)
    Y = adj @ (adj.T @ (adj @ G))
    Qb, _ = np.linalg.qr(Y)
    Ur = np.ascontiguousarray(Qb[:, :RCORR])                  # (1600, 64)
    eps = wxt - wxt8.astype(f32) / f32(S_X)                   # (B,1600,440)
    ptv = np.einsum('kr,bkm->brm', Ur, eps, optimize=True)    # (B,64,440)
    # corr rows 0:64 = S_X * Ur^T eps ; rows 64:128 = S_X * h[1536:1600]
    pt = np.zeros((B, KCORR, RQ), np.float16)
    pt[:, :RCORR] = (ptv * f32(S_X)).astype(np.float16)
    pt[:, RCORR:] = (wxt[:, 1536:1600] * f32(S_X)).astype(np.float16)
    cr = np.zeros((KCORR, N2), np.float16)
    cr[:RCORR] = (Ur.T @ adj * f32(S_A)).astype(np.float16)
    cr[RCORR:] = (adj[1536:1600] * f32(S_A)).astype(np.float16)

    def wbig(W):
        Wb = np.zeros((Q, RQ), f32)
        W0, W1 = np.asarray(W[:, :, 0], f32), np.asarray(W[:, :, 1], f32)
        for r in range(R):
            Wb[r * C:(r + 1) * C, r * C:(r + 1) * C] = W0.T
            Wb[(r + 1) * C:(r + 2) * C, r * C:(r + 1) * C] = W1.T
        return Wb

    # pairs: even (0,1),(2,3) + odd (1,2)
    wbig_offs = (0, 240, 120)
    wbigpf = _pack_pairs(_q8(wbig(inp['W_f']), S_W), 120, 3, MPAD, wbig_offs)
    wbigpg = _pack_pairs(_q8(wbig(inp['W_g']), S_W), 120, 3, MPAD, wbig_offs)

    wres = np.zeros((Q, RQ), f32)
    eye = np.eye(C, dtype=f32)
    Wr = np.asarray(inp['W_res'], f32) * BN_SCALE             # (11,12)
    for t in range(T):
        for r in range(R):
            wres[t * C:(t + 1) * C, r * C:(r + 1) * C] = Wr[r, t] * eye
    wresp = _pack_pairs(_q8(wres, S_W), 120, 2, MPAD)

    # skip weights fp16, no BN (rhs oraw' already carries BN)
    wskip = np.zeros((RQ, SQ), f32)
    Ws = np.asarray(inp['W_skip'], f32)                       # (12,11)
    bs = np.asarray(inp['b_skip'], f32) * BN_SCALE
    for s in range(12):
        for r in range(R):
            wskip[r * C:(r + 1) * C, s * C:(s + 1) * C] = Ws[s, r] * eye
    wskip16 = wskip.astype(np.float16)
    wskip_bias = np.zeros((1, SQ), np.float16)
    for s in range(12):
        wskip_bias[0, s * C:(s + 1) * C] = bs[s]

    def blkdiag3(A):
        M_ = np.zeros((120, 120), f32)
        for j in range(3):
            M_[j * C:(j + 1) * C, j * C:(j + 1) * C] = A
        return M_

    wmix1 = (blkdiag3(np.asarray(inp['W_gcn'][0], f32).T)
             * f32(S_H1 / (S_X * S_A))).astype(np_bf16)
    wmix2 = (blkdiag3(np.asarray(inp['W_gcn'][1], f32).T)
             * f32(BN_SCALE / (S_H1 * S_A))).astype(np_bf16)

    bias_f = np.ascontiguousarray(np.tile(np.asarray(inp['b_f'], f32), 3)[:, None])
    bias_g = np.ascontiguousarray(np.tile(np.asarray(inp['b_g'], f32), 3)[:, None])

    bres = np.asarray(inp['b_res'], f32) * BN_SCALE
    bres_tile = np.zeros((120, 1), f32)
    for p in range(120):
        r = p // C
        bres_tile[p, 0] = bres[r] if r < R else 0.0

    shared = dict(adjp=adjp, adjt=adjt, cr=cr,
                  wbigpf=wbigpf.reshape(3 * 120, 2, MPAD),
                  wbigpg=wbigpg.reshape(3 * 120, 2, MPAD),
                  wresp=wresp.reshape(2 * 120, 2, MPAD),
                  wskip=wskip16, wskip_bias=wskip_bias,
                  wmix1=wmix1, wmix2=wmix2,
                  bias_f=bias_f, bias_g=bias_g, bres=bres_tile)
    per_core = []
    for c_ in range(NCORES):
        sl = slice(c_ * BL, (c_ + 1) * BL)
        ptc = np.zeros((KCORR, BL * RQ), np.float16)
        for b in range(BL):
            ptc[:, b * RQ:(b + 1) * RQ] = pt[c_ * BL + b]
        per_core.append(dict(
            wxtp=np.ascontiguousarray(wxtp[sl]).reshape(BL * NPAIR, 128, 2, MPAD),
            xp8p=np.ascontiguousarray(xp8p[sl]).reshape(BL * 3, 120, 2, N),
            pt=ptc))
    has_bres = bool(np.any(bres))
    has_bskip = bool(np.any(bs))
    return shared, per_core, has_bres, has_bskip


# ---------------------------------------------------------------------------
# device program
# ---------------------------------------------------------------------------

def _build_program(has_bres, has_bskip):
    nc = bacc.Bacc("TRN2", target_bir_lowering=False, debug=False,
                   enable_asserts=False, num_devices=NCORES)

    wxtp_d = nc.dram_tensor("wxtp", [BL * NPAIR, 128, 2, MPAD], F8,
                            kind="ExternalInput").ap()
    xp8p_d = nc.dram_tensor("xp8p", [BL * 3, 120, 2, N], F8,
                            kind="ExternalInput").ap()
    pt_d = nc.dram_tensor("pt", [KCORR, BL * RQ], F16, kind="ExternalInput").ap()
    adjp_d = nc.dram_tensor("adjp", [NPAIR, 128, 2, N2], F8,
                            kind="ExternalInput").ap()
    adjt_d = nc.dram_tensor("adjt", [64, N2], F8, kind="ExternalInput").ap()
    cr_d = nc.dram_tensor("cr", [KCORR, N2], F16, kind="ExternalInput").ap()
    wbigpf_d = nc.dram_tensor("wbigpf", [360, 2, MPAD], F8,
                              kind="ExternalInput").ap()
    wbigpg_d = nc.dram_tensor("wbigpg", [360, 2, MPAD], F8,
                              kind="ExternalInput").ap()
    wresp_d = nc.dram_tensor("wresp", [240, 2, MPAD], F8,
                             kind="ExternalInput").ap()
    wskip_d = nc.dram_tensor("wskip", [RQ, SQ], F16, kind="ExternalInput").ap()
    wskipb_d = nc.dram_tensor("wskip_bias", [1, SQ], F16,
                              kind="ExternalInput").ap()
    wmix1_d = nc.dram_tensor("wmix1", [120, 120], BF16, kind="ExternalInput").ap()
    wmix2_d = nc.dram_tensor("wmix2", [120, 120], BF16, kind="ExternalInput").ap()
    biasf_d = nc.dram_tensor("bias_f", [120, 1], F32, kind="ExternalInput").ap()
    biasg_d = nc.dram_tensor("bias_g", [120, 1], F32, kind="ExternalInput").ap()
    bres_d = nc.dram_tensor("bres", [120, 1], F32, kind="ExternalInput").ap()
    # output rows per batch: 0:440 final (r,c), 440:920 skip (s,c)
    out_d = nc.dram_tensor("out", [BL, 920, N], F16, kind="ExternalOutput").ap()

    with tile.TileContext(nc) as tc:
        _emit(nc, tc, dict(wxtp=wxtp_d, xp8p=xp8p_d, pt=pt_d,
                           adjp=adjp_d, adjt=adjt_d, cr=cr_d, wbigpf=wbigpf_d,
                           wbigpg=wbigpg_d, wresp=wresp_d, wskip=wskip_d,
                           wskipb=wskipb_d, wmix1=wmix1_d, wmix2=wmix2_d,
                           biasf=biasf_d, biasg=biasg_d, bres=bres_d,
                           out=out_d),
              has_bres, has_bskip)
    nc.compile()
    return nc


def _emit(nc, tc, d, has_bres, has_bskip):
    from contextlib import ExitStack
    AF = mybir.ActivationFunctionType
    ALU = mybir.AluOpType
    DR = mybir.MatmulPerfMode.DoubleRow
    DESC_G = float(1.0 / (S_X * S_W))          # gate pre-activation descale
    DESC_R = float(1.0 / (S_X * S_W))          # res psum descale (BN in wres)

    ctx = ExitStack()
    with ctx:
        const = ctx.enter_context(tc.tile_pool(name="const", bufs=1))
        xp8_p = ctx.enter_context(tc.tile_pool(name="xp8", bufs=4))
        wxt_p = ctx.enter_context(tc.tile_pool(name="wxt", bufs=2))
        dres_p = ctx.enter_context(tc.tile_pool(name="dres", bufs=3))
        h0_p = ctx.enter_context(tc.tile_pool(name="h0", bufs=2))
        h1t_p = ctx.enter_context(tc.tile_pool(name="h1t", bufs=2))
        h2_p = ctx.enter_context(tc.tile_pool(name="h2", bufs=2))
        oraw_p = ctx.enter_context(tc.tile_pool(name="oraw", bufs=2))
        tmp_p = ctx.enter_context(tc.tile_pool(name="tmp", bufs=2))
        fin_p = ctx.enter_context(tc.tile_pool(name="fin", bufs=4))
        psA = ctx.enter_context(tc.tile_pool(name="psA", bufs=6, space="PSUM"))
        psB = ctx.enter_context(tc.tile_pool(name="psB", bufs=2, space="PSUM"))

        # ---- DMA: tconv-critical first ----
        biasf_sb = const.tile([120, 1], F32, name="biasf")
        nc.sync.dma_start(biasf_sb[:], d['biasf'][:])
        biasg_sb = const.tile([120, 1], F32, name="biasg")
        nc.scalar.dma_start(biasg_sb[:], d['biasg'][:])
        wbig_sb = {}
        for gname, wd in (("f", d['wbigpf']), ("g", d['wbigpg'])):
            tiles = []
            for p in range(3):
                t = const.tile([120, 2, MPAD], F8, name=f"wbigp{gname}{p}")
                eng = nc.sync if gname == "f" else nc.scalar
                eng.dma_start(t[:, :, :], wd[120 * p:120 * (p + 1), :, :])
                tiles.append(t)
            wbig_sb[gname] = tiles

        def load_xp8(b):
            xp8_sb = []
            for p in range(3):
                t = xp8_p.tile([120, 2, N], F8, name=f"xp8p{p}",
                               tag=f"xp8p{p}", bufs=4)
                eng = (nc.sync, nc.scalar, nc.gpsimd)[p]
                eng.dma_start(t[:, :, :], d['xp8p'][b * 3 + p, :, :, :])
                xp8_sb.append(t)
            return xp8_sb

        def load_wxt(b):
            wxtp_sb = []
            for j in range(NPAIR):
                t = wxt_p.tile([128, 2, MPAD], F8, name=f"wxtp{j}",
                               tag=f"wxtp{j}", bufs=2)
                eng = nc.sync if j % 2 == 0 else nc.scalar
                eng.dma_start(t[:, :, :], d['wxtp'][b * NPAIR + j, :, :, :])
                wxtp_sb.append(t)
            return wxtp_sb

        xp8_all = [load_xp8(0), load_xp8(1)]

        # correction operands + adjacency; adj in column-chunk-major order so
        # hop0's first chunk starts after ~1/4 of the adj bytes
        pt_sb = const.tile([KCORR, BL * RQ], F16, name="pt")
        nc.sync.dma_start(pt_sb[:], d['pt'][:])
        cr_sb = const.tile([KCORR, N2], F16, name="cr")
        nc.scalar.dma_start(cr_sb[:], d['cr'][:])
        adjp_sb = [const.tile([128, 2, N2], F8, name=f"adjp{j}")
                   for j in range(NPAIR)]
        adjt_sb = const.tile([64, N2], F8, name="adjt")
        for (co, cs) in CH1600:
            for j in range(NPAIR):
                nc.gpsimd.dma_start(adjp_sb[j][:, :, co:co + cs],
                                    d['adjp'][j, :, :, co:co + cs])
            nc.gpsimd.dma_start(adjt_sb[:, co:co + cs],
                                d['adjt'][:, co:co + cs])

        wxt_0 = load_wxt(0)

        wmix1_sb = const.tile([120, 120], BF16, name="wmix1")
        nc.sync.dma_start(wmix1_sb[:], d['wmix1'][:])
        wmix2_sb = const.tile([120, 120], BF16, name="wmix2")
        nc.sync.dma_start(wmix2_sb[:], d['wmix2'][:])
        wskip_sb = []
        for k, (o, s) in enumerate(KS_SKIP):
            t = const.tile([s, SQ], F16, name=f"wskip{k}")
            nc.sync.dma_start(t[:], d['wskip'][o:o + s, :])
            wskip_sb.append(t)
        if has_bskip:
            wskipb_sb = const.tile([1, SQ], F16, name="wskipb")
            nc.sync.dma_start(wskipb_sb[:], d['wskipb'][:])
            ones_sb = const.tile([1, N], F16, name="ones")
            nc.vector.memset(ones_sb[:], 1.0)
        wresp_sb = []
        for p in range(2):
            t = const.tile([120, 2, MPAD], F8, name=f"wresp{p}")
            nc.sync.dma_start(t[:, :, :], d['wresp'][120 * p:120 * (p + 1), :, :])
            wresp_sb.append(t)
        bres_sb = const.tile([120, 1], F32, name="bres_t")
        nc.sync.dma_start(bres_sb[:], d['bres'][:])

        def tconv_b(b, xp8_sb):
            dres_sb = []
            for m, (mo, ms) in enumerate(M_BLOCKS):
                dr_t = dres_p.tile([120, N], F16, name=f"dres{m}",
                                   tag=f"dres{m}", bufs=3)
                dres_sb.append(dr_t)
                gate_sb = {}
                for gname, bias_sb in (("f", biasf_sb), ("g", biasg_sb)):
                    wp = wbig_sb[gname]
                    for (co, cs) in CH800:
                        ps = psA.tile([120, 400], F32, name="tc_ps", tag="psA")
                        if m < 3:
                            # pair index: m=0 -> even0, m=1 -> odd, m=2 -> even1
                            pi = (0, 2, 1)[m]
                            nc.tensor.matmul(
                                ps[0:ms, :], wp[pi][:, :, mo:mo + ms],
                                xp8_sb[pi][:, :, co:co + cs],
                                start=True, stop=True, perf_mode=DR)
                        else:
                            nc.tensor.matmul(
                                ps[0:ms, :], wp[1][:, 1, mo:mo + ms],
                                xp8_sb[1][:, 1, co:co + cs],
                                start=True, stop=True)
                        g = tmp_p.tile([120, 400], F16, name=f"g{gname}",
                                       tag=f"gate{gname}{co}", bufs=2)
                        nc.scalar.activation(
                            g[0:ms, :], ps[0:ms, :],
                            AF.Tanh if gname == "f" else AF.Sigmoid,
                            bias=bias_sb[0:ms, :], scale=DESC_G)
                        gate_sb[(gname, co)] = g
                for (co, cs) in CH800:
                    # dres' = BN * tanh * sigmoid
                    nc.vector.scalar_tensor_tensor(
                        dr_t[0:ms, co:co + cs], gate_sb[("f", co)][0:ms, :],
                        BN_SCALE, gate_sb[("g", co)][0:ms, :],
                        op0=ALU.mult, op1=ALU.mult)
            return dres_sb

        def hops_b(b, wxtp_sb, dres_sb):
            # hop0 + rank correction + fp16 K-tail -> h0 (bf16, S_X*S_A scaled)
            h0_sb = []
            for m, (mo, ms) in enumerate(M_BLOCKS):
                h0 = h0_p.tile([120, N2], BF16, name=f"h0_{m}", tag=f"h0_{m}",
                               bufs=2)
                h0_sb.append(h0)
                for (co, cs) in CH1600:
                    ps = psA.tile([120, 400], F32, name="h0_ps", tag="psA")
                    for j in range(NPAIR):
                        nc.tensor.matmul(ps[0:ms, :],
                                         wxtp_sb[j][:, :, mo:mo + ms],
                                         adjp_sb[j][:, :, co:co + cs],
                                         start=(j == 0), stop=False,
                                         perf_mode=DR)
                    nc.tensor.matmul(ps[0:ms, :],
                                     pt_sb[:, b * RQ + mo: b * RQ + mo + ms],
                                     cr_sb[:, co:co + cs],
                                     start=False, stop=True)
                    nc.vector.tensor_copy(h0[0:ms, co:co + cs], ps[0:ms, :])
            # mix1 -> h1t fp8 paired (nodes on partitions), scaled by S_H1;
            # all four m-blocks land in one PSUM tile -> one relu per node-tile
            h1tp_sb = [h1t_p.tile([128, 2, MPAD], F8, name=f"h1tp{j}",
                                  tag=f"h1tp{j}", bufs=2) for j in range(NPAIR)]
            h1tt_sb = h1t_p.tile([64, RQ], F8, name="h1tt", tag="h1tt", bufs=2)
            for i, (o, s) in enumerate(N2_SPLIT):
                bp = psB.tile([128, RQ], F32, name="m1_ps", tag="psB")
                # four disjoint-column writes into one zeroed PSUM bank:
                # start=True zeroes the whole 2KB bank, the rest accumulate
                for m, (mo, ms) in enumerate(M_BLOCKS):
                    nc.tensor.matmul(bp[0:s, mo:mo + ms],
                                     h0_sb[m][0:ms, o:o + s],
                                     wmix1_sb[0:ms, 0:ms],
                                     start=(m == 0), stop=(m == 3),
                                     skip_group_check=True)
                if i < 12:
                    nc.vector.tensor_relu(h1tp_sb[i // 2][0:s, i % 2, 0:RQ],
                                          bp[0:s, :])
                else:
                    nc.vector.tensor_relu(h1tt_sb[0:s, :], bp[0:s, :])
            # hop1 -> h2 (bf16, scaled by S_H1*S_A)
            h2_sb = []
            for m, (mo, ms) in enumerate(M_BLOCKS):
                h2 = h2_p.tile([120, N], BF16, name=f"h2_{m}", tag=f"h2_{m}",
                               bufs=2)
                h2_sb.append(h2)
                for (co, cs) in CH800:
                    ps = psA.tile([120, 400], F32, name="h1_ps", tag="psA")
                    for j in range(NPAIR):
                        nc.tensor.matmul(ps[0:ms, :],
                                         h1tp_sb[j][:, :, mo:mo + ms],
                                         adjp_sb[j][:, :, 800 + co:800 + co + cs],
                                         start=(j == 0), stop=False,
                                         perf_mode=DR)
                    nc.tensor.matmul(ps[0:ms, :], h1tt_sb[:, mo:mo + ms],
                                     adjt_sb[:, 800 + co:800 + co + cs],
                                     start=False, stop=True)
                    nc.scalar.copy(h2[0:ms, co:co + cs], ps[0:ms, :])
            # mix2 + relu (+BN via wmix2) + dres -> oraw' (fp16)
            oraw_sb = []
            for m, (mo, ms) in enumerate(M_BLOCKS):
                orw = oraw_p.tile([120, N], F16, name=f"oraw{m}", tag=f"oraw{m}",
                                  bufs=2)
                oraw_sb.append(orw)
                for (co, cs) in CH800:
                    ps = psA.tile([120, 400], F32, name="m2_ps", tag="psA")
                    nc.tensor.matmul(ps[0:ms, :], wmix2_sb[0:ms, 0:ms],
                                     h2_sb[m][0:ms, co:co + cs],
                                     start=True, stop=True)
                    rl = tmp_p.tile([120, 400], F16, name="rl", tag=f"rl{co}",
                                    bufs=2)
                    nc.scalar.activation(rl[0:ms, :], ps[0:ms, :], AF.Relu)
                    nc.vector.tensor_add(orw[0:ms, co:co + cs], rl[0:ms, :],
                                         dres_sb[m][0:ms, co:co + cs])
            return oraw_sb

        def epilogue_b(b, xp8_sb, oraw_sb):
            # skip -> out rows 440:920  (skip = wskip^T @ oraw', fp16)
            for sm in range(4):
                for (co, cs) in CH800:
                    ps = psA.tile([120, 400], F32, name="sk_ps", tag="psA")
                    for kt, (o, s) in enumerate(KS_SKIP):
                        nc.tensor.matmul(
                            ps[:, :],
                            wskip_sb[kt][:, sm * 120:(sm + 1) * 120],
                            oraw_sb[kt][0:s, co:co + cs],
                            start=(kt == 0),
                            stop=(kt == 3 and not has_bskip))
                    if has_bskip:
                        nc.tensor.matmul(ps[:, :],
                                         wskipb_sb[:, sm * 120:(sm + 1) * 120],
                                         ones_sb[:, co:co + cs],
                                         start=False, stop=True)
                    sk = fin_p.tile([120, 400], F16, name="sk", tag="sk", bufs=3)
                    nc.scalar.copy(sk[:, :], ps[:, :])
                    nc.scalar.dma_start(
                        d['out'][b, RQ + sm * 120:RQ + (sm + 1) * 120,
                                 co:co + cs], sk[:, :])
            # residual (fp8 DR) + final -> out rows 0:440
            for m, (mo, ms) in enumerate(M_BLOCKS):
                for (co, cs) in CH800:
                    ps = psA.tile([120, 400], F32, name="rs_ps", tag="psA")
                    for p in range(2):
                        nc.tensor.matmul(ps[0:ms, :],
                                         wresp_sb[p][:, :, mo:mo + ms],
                                         xp8_sb[p][:, :, co:co + cs],
                                         start=(p == 0), stop=(p == 1),
                                         perf_mode=DR)
                    fin = fin_p.tile([120, 400], F16, name="fin", tag="fin",
                                     bufs=3)
                    nc.vector.scalar_tensor_tensor(
                        fin[0:ms, :], ps[0:ms, :], DESC_R,
                        oraw_sb[m][0:ms, co:co + cs],
                        op0=ALU.mult, op1=ALU.add)
                    if has_bres:
                        nc.vector.tensor_scalar_add(fin[0:ms, :], fin[0:ms, :],
                                                    bres_sb[0:ms, :])
                    nc.scalar.dma_start(d['out'][b, mo:mo + ms, co:co + cs],
                                        fin[0:ms, :])

        # pipeline: tconv(0), tconv(1) first; then hops(b)/epi(b-1)/tconv(b+2)
        dres_all = [tconv_b(0, xp8_all[0]), tconv_b(1, xp8_all[1])]
        wxt_all = [wxt_0]
        prev = None
        for b in range(BL):
            if b + 1 < BL:
                wxt_all.append(load_wxt(b + 1))
            if b + 2 < BL:
                xp8_all.append(load_xp8(b + 2))
                dres_all.append(tconv_b(b + 2, xp8_all[b + 2]))
            if prev is not None:
                epilogue_b(*prev)
            oraw_sb = hops_b(b, wxt_all[b], dres_all[b])
            prev = (b, xp8_all[b], oraw_sb)
        epilogue_b(*prev)


_CACHE = {}


def kernel(**inputs):
    shared, per_core, has_bres, has_bskip = _prep(inputs)

    key = ("prog", has_bres, has_bskip)
    if key not in _CACHE:
        _CACHE[key] = _build_program(has_bres, has_bskip)
    nc = _CACHE[key]

    in_maps = []
    for core in range(NCORES):
        m = dict(shared)
        m.update(per_core[core])
        in_maps.append(m)

    import os
    trace = bool(int(os.environ.get("KERNEL_TRACE", "0")))
    res = run_bass_kernel_spmd(nc, in_maps, core_ids=list(range(NCORES)),
                               trace=trace)
    kernel.last_result = res
    outs = [np.asarray(r["out"], np.float32) for r in res.results]
    full = np.concatenate(outs, axis=0)               # (32, 920, 800)
    full = full.reshape(B, 23, C, N).transpose(0, 2, 1, 3)   # (B, C, 23, N)
    return np.ascontiguousarray(full)
